# revision 3
# baseline (speedup 1.0000x reference)
"""Trainium2 Bass kernel for nn_ComplexFaberConv (gnn_message_passing).

Strategy
--------
Host algebra: the K-hop einsum collapses (sum_k s_k W[k] -> one 128x128
effective weight per real/imag), and the degree normalization factorizes as
val_e = a[dst] * b[src].  Everything reduces to a pure gather + segment-sum
over a [2N, 256] feature table (features x effective weights x src-side
degree factors; real||imag concat).

The wire (axon RPC tunnel, ~40MB/s) dominates wall time, so the table is
built ON DEVICE instead of being uploaded: each core receives only its bf16
transposed x shard (6.4MB), computes its table shard with 5 matmuls per
128-node tile, and an 8-core AllGather replicates the full bf16 table into
device DRAM.  Phase 2 is the gather + segment-sum: for each 128-node dst
tile, gather the tile's edges in 128-edge chunks (indirect DMA), build a
selection matrix sel[e, d] = (dst_slot[e] == d) with one DVE is_equal, and
accumulate psum[128 dst, 256] += sel.T @ gathered on the tensor engine.
Outputs go back as bf16.  Host un-permutes and adds the bias row.
"""
import numpy as np
import ml_dtypes

import concourse.bass as bass
import concourse.bacc as bacc
import concourse.mybir as mybir
import concourse.tile as tile
from concourse import bass_utils

K = 3
ALPHA = 0.5
EXPONENT = -0.25
NCORES = 8
P = 128
DCAT = 256          # real||imag feature width
N = 100000
TPC = -(-N // (NCORES * P))   # 98 node tiles per core
NPC = TPC * P                 # 12544 nodes per core (padded)
NPAD = NCORES * NPC

# set by tests to run CoreSim instead of hardware
_SIM = False

_prog_cache = {}
_last_info = {}


# --------------------------------------------------------------------------
# host-side preparation
# --------------------------------------------------------------------------

def _host_prep(x_real, x_imag, W_real, W_imag, b_real, b_imag, edge_index):
    n = x_real.shape[0]
    assert n == N
    row = edge_index[0].astype(np.int64)
    col = edge_index[1].astype(np.int64)
    tpc = TPC
    nbins = NCORES * tpc

    deg_out = np.bincount(row, minlength=n).astype(np.float32)
    deg_in = np.bincount(col, minlength=n).astype(np.float32)
    with np.errstate(divide="ignore"):
        afull = np.where(deg_out > 0, deg_out ** np.float32(EXPONENT), 0.0)
        bfull = np.where(deg_in > 0, deg_in ** np.float32(EXPONENT), 0.0)
    afull = afull.astype(np.float32)
    bfull = bfull.astype(np.float32)

    s = (0.5 ** np.arange(K)).astype(np.float32)
    Wr = np.einsum("kod,k->od", W_real, s).astype(np.float32)
    Wi = np.einsum("kod,k->od", W_imag, s).astype(np.float32)
    c1 = (s @ b_real - s @ b_imag).astype(np.float32)
    c2 = (s @ b_real + s @ b_imag).astype(np.float32)

    # device weights: [d, o] layout (matmul rhs), pre-scaled
    wA = (0.5 * Wr.T).astype(ml_dtypes.bfloat16)
    wB = (-0.5 * Wi.T).astype(ml_dtypes.bfloat16)
    wC = Wi.T.astype(ml_dtypes.bfloat16)

    # transposed, padded, bf16 x shards: [128 feat, NPC nodes] per core
    xrT = np.zeros((P, NPAD), dtype=ml_dtypes.bfloat16)
    xiT = np.zeros((P, NPAD), dtype=ml_dtypes.bfloat16)
    xrT[:, :n] = x_real.T
    xiT[:, :n] = x_imag.T

    # phase-1 src-side scale vectors, natural node order: [P, TPC] per core
    apad = np.zeros(NPAD, dtype=np.float32)
    bpad = np.zeros(NPAD, dtype=np.float32)
    apad[:n] = afull
    bpad[:n] = bfull
    avec = apad.reshape(NCORES, TPC, P).transpose(0, 2, 1).copy()
    bvec = bpad.reshape(NCORES, TPC, P).transpose(0, 2, 1).copy()

    # ---- balance nodes into (core, tile) bins: sorted round-robin on degree
    load = deg_out + deg_in
    order = np.argsort(-load, kind="stable")
    idx = np.arange(n)
    node_bin = np.empty(n, dtype=np.int64)
    node_slot = np.empty(n, dtype=np.int64)
    node_bin[order] = idx % nbins
    node_slot[order] = idx // nbins
    gslot = (node_bin // tpc) * NPC + (node_bin % tpc) * P + node_slot
    core_of = node_bin // tpc
    tile_of = node_bin % tpc

    fwd_cnt = np.bincount(node_bin[row], minlength=nbins)
    bwd_cnt = np.bincount(node_bin[col], minlength=nbins)
    cf = int(-(-fwd_cnt.max() // P))
    cb = int(-(-bwd_cnt.max() // P))
    cpt = cf + cb
    nch = tpc * cpt

    src_all = np.zeros((NCORES, P, nch), dtype=np.int32)
    dstf_all = np.full((NCORES, P, nch), -1.0, dtype=np.float32)
    for direction in range(2):
        dst = row if direction == 0 else col
        src = col if direction == 0 else row
        tabrow = (src // NPC) * (2 * NPC) + (src % NPC) + (0 if direction == 0 else NPC)
        dbin = node_bin[dst]
        eorder = np.argsort(dbin, kind="stable")
        dbin_s = dbin[eorder]
        slot_s = node_slot[dst][eorder]
        tab_s = tabrow[eorder]
        starts = np.searchsorted(dbin_s, np.arange(nbins + 1))
        r = np.arange(dst.shape[0]) - starts[dbin_s]
        cbase = 0 if direction == 0 else cf
        colidx = (dbin_s % tpc) * cpt + cbase + r // P
        corei = dbin_s // tpc
        src_all[corei, r % P, colidx] = tab_s
        dstf_all[corei, r % P, colidx] = slot_s

    afac = np.zeros((NCORES, P, tpc), dtype=np.float32)
    bfac = np.zeros((NCORES, P, tpc), dtype=np.float32)
    afac[core_of, node_slot, tile_of] = afull
    bfac[core_of, node_slot, tile_of] = bfull

    iota = np.broadcast_to(np.arange(P, dtype=np.float32), (P, P)).copy()

    return dict(xrT=xrT, xiT=xiT, wA=wA, wB=wB, wC=wC, avec=avec, bvec=bvec,
                src_all=src_all, dstf_all=dstf_all, afac=afac, bfac=bfac,
                c1=c1, c2=c2, gslot=gslot, cf=cf, cb=cb, tpc=tpc, n=n,
                iota=iota)


# --------------------------------------------------------------------------
# device program
# --------------------------------------------------------------------------

def _build_program(cf, cb, tpc):
    cpt = cf + cb
    nch = tpc * cpt
    nc = bacc.Bacc("TRN2", target_bir_lowering=False, debug=False,
                   num_devices=NCORES)
    f32 = mybir.dt.float32
    bf16 = mybir.dt.bfloat16
    i32 = mybir.dt.int32
    xrT = nc.dram_tensor("xrT", [P, NPC], bf16, kind="ExternalInput").ap()
    xiT = nc.dram_tensor("xiT", [P, NPC], bf16, kind="ExternalInput").ap()
    wA = nc.dram_tensor("wA", [P, P], bf16, kind="ExternalInput").ap()
    wB = nc.dram_tensor("wB", [P, P], bf16, kind="ExternalInput").ap()
    wC = nc.dram_tensor("wC", [P, P], bf16, kind="ExternalInput").ap()
    avec = nc.dram_tensor("avec", [P, TPC], f32, kind="ExternalInput").ap()
    bvec = nc.dram_tensor("bvec", [P, TPC], f32, kind="ExternalInput").ap()
    srcs = nc.dram_tensor("srcs", [P, nch], i32, kind="ExternalInput").ap()
    dstf = nc.dram_tensor("dstf", [P, nch], f32, kind="ExternalInput").ap()
    afac = nc.dram_tensor("afac", [P, tpc], f32, kind="ExternalInput").ap()
    bfac = nc.dram_tensor("bfac", [P, tpc], f32, kind="ExternalInput").ap()
    iota = nc.dram_tensor("iota", [P, P], f32, kind="ExternalInput").ap()
    out = nc.dram_tensor("out", [tpc * P, DCAT], bf16, kind="ExternalOutput").ap()

    with tile.TileContext(nc) as tc:
        with (
            tc.tile_pool(name="meta", bufs=1) as meta_tp,
            tc.tile_pool(name="gtab", bufs=4) as gtab_tp,
            tc.tile_pool(name="g", bufs=8) as g_tp,
            tc.tile_pool(name="sel", bufs=8) as sel_tp,
            tc.tile_pool(name="post", bufs=3) as post_tp,
            tc.tile_pool(name="ps1", bufs=1, space="PSUM") as ps1_tp,
            tc.tile_pool(name="ps2", bufs=2, space="PSUM") as ps2_tp,
            tc.tile_pool(name="dram", bufs=1, space="DRAM") as dram_tp,
        ):
            xr_sb = meta_tp.tile([P, NPC], bf16)
            nc.sync.dma_start(out=xr_sb[:], in_=xrT[:])
            xi_sb = meta_tp.tile([P, NPC], bf16)
            nc.sync.dma_start(out=xi_sb[:], in_=xiT[:])
            wA_sb = meta_tp.tile([P, P], bf16)
            nc.sync.dma_start(out=wA_sb[:], in_=wA[:])
            wB_sb = meta_tp.tile([P, P], bf16)
            nc.sync.dma_start(out=wB_sb[:], in_=wB[:])
            wC_sb = meta_tp.tile([P, P], bf16)
            nc.sync.dma_start(out=wC_sb[:], in_=wC[:])
            avec_sb = meta_tp.tile([P, TPC], f32)
            nc.sync.dma_start(out=avec_sb[:], in_=avec[:])
            bvec_sb = meta_tp.tile([P, TPC], f32)
            nc.sync.dma_start(out=bvec_sb[:], in_=bvec[:])
            srcs_sb = meta_tp.tile([P, nch], i32)
            nc.sync.dma_start(out=srcs_sb[:], in_=srcs[:])
            dstf_sb = meta_tp.tile([P, nch], f32)
            nc.sync.dma_start(out=dstf_sb[:], in_=dstf[:])
            afac_sb = meta_tp.tile([P, tpc], f32)
            nc.sync.dma_start(out=afac_sb[:], in_=afac[:])
            bfac_sb = meta_tp.tile([P, tpc], f32)
            nc.sync.dma_start(out=bfac_sb[:], in_=bfac[:])
            iota_sb = meta_tp.tile([P, P], f32)
            nc.sync.dma_start(out=iota_sb[:], in_=iota[:])

            tab_local = dram_tp.tile([2 * NPC, DCAT], bf16)
            tab_full = dram_tp.tile([NCORES * 2 * NPC, DCAT], bf16)

            # ---- phase 1: build this core's table shard
            for t in range(TPC):
                xr_t = xr_sb[:, t * P:(t + 1) * P]
                xi_t = xi_sb[:, t * P:(t + 1) * P]
                psH = ps1_tp.tile([P, P], f32, space="PSUM", tag="psH")
                psI1 = ps1_tp.tile([P, P], f32, space="PSUM", tag="psI1")
                psI2 = ps1_tp.tile([P, P], f32, space="PSUM", tag="psI2")
                nc.tensor.matmul(out=psH[:], lhsT=xr_t, rhs=wA_sb[:],
                                 start=True, stop=False)
                nc.tensor.matmul(out=psH[:], lhsT=xi_t, rhs=wB_sb[:],
                                 start=False, stop=True)
                nc.tensor.matmul(out=psI1[:], lhsT=xr_t, rhs=wC_sb[:],
                                 start=True, stop=False)
                nc.tensor.matmul(out=psI1[:], lhsT=xi_t, rhs=wA_sb[:],
                                 start=False, stop=True)
                nc.tensor.matmul(out=psI2[:], lhsT=xi_t, rhs=wA_sb[:],
                                 start=True, stop=True)
                gf = gtab_tp.tile([P, DCAT], bf16, tag="gf")
                nc.scalar.activation(
                    out=gf[:, :P], in_=psH[:],
                    func=mybir.ActivationFunctionType.Copy,
                    scale=bvec_sb[:, t:t + 1])
                nc.scalar.activation(
                    out=gf[:, P:], in_=psI1[:],
                    func=mybir.ActivationFunctionType.Copy,
                    scale=bvec_sb[:, t:t + 1])
                gb = gtab_tp.tile([P, DCAT], bf16, tag="gb")
                nc.scalar.activation(
                    out=gb[:, :P], in_=psH[:],
                    func=mybir.ActivationFunctionType.Copy,
                    scale=avec_sb[:, t:t + 1])
                nc.scalar.activation(
                    out=gb[:, P:], in_=psI2[:],
                    func=mybir.ActivationFunctionType.Copy,
                    scale=avec_sb[:, t:t + 1])
                nc.sync.dma_start(out=tab_local[t * P:(t + 1) * P], in_=gf[:])
                nc.sync.dma_start(out=tab_local[NPC + t * P:NPC + (t + 1) * P],
                                  in_=gb[:])

            # ---- replicate the table across cores
            nc.gpsimd.collective_compute(
                "AllGather",
                mybir.AluOpType.bypass,
                replica_groups=[list(range(NCORES))],
                ins=[tab_local[:].opt()],
                outs=[tab_full[:].opt()],
            )

            # ---- phase 2: gather + segment-sum over balanced dst tiles
            for t in range(tpc):
                pf = ps2_tp.tile([P, DCAT], f32, space="PSUM", tag="pf")
                pb = ps2_tp.tile([P, DCAT], f32, space="PSUM", tag="pb")
                for c in range(cpt):
                    colx = t * cpt + c
                    gt = g_tp.tile([P, DCAT], bf16, tag="gt")
                    nc.gpsimd.indirect_dma_start(
                        out=gt[:], out_offset=None, in_=tab_full[:],
                        in_offset=bass.IndirectOffsetOnAxis(
                            ap=srcs_sb[:, colx:colx + 1], axis=0))
                    sel = sel_tp.tile([P, P], bf16, tag="sel")
                    nc.vector.tensor_tensor(
                        out=sel[:],
                        in0=dstf_sb[:, colx:colx + 1].to_broadcast([P, P]),
                        in1=iota_sb[:],
                        op=mybir.AluOpType.is_equal)
                    tgt = pf if c < cf else pb
                    nc.tensor.matmul(
                        out=tgt[:], lhsT=sel[:], rhs=gt[:],
                        start=(c == 0 or c == cf),
                        stop=(c == cf - 1 or c == cpt - 1))
                s1 = post_tp.tile([P, DCAT], f32, tag="s1")
                nc.scalar.activation(
                    out=s1[:], in_=pf[:],
                    func=mybir.ActivationFunctionType.Copy,
                    scale=afac_sb[:, t:t + 1])
                s2 = post_tp.tile([P, DCAT], f32, tag="s2")
                nc.vector.tensor_scalar_mul(
                    out=s2[:], in0=pb[:], scalar1=bfac_sb[:, t:t + 1])
                ot = post_tp.tile([P, DCAT], bf16, tag="ot")
                nc.vector.tensor_tensor(
                    out=ot[:], in0=s1[:], in1=s2[:], op=mybir.AluOpType.add)
                nc.sync.dma_start(out=out[t * P:(t + 1) * P], in_=ot[:])
    nc.compile()
    return nc


def _get_program(cf, cb, tpc):
    key = (cf, cb, tpc)
    if key not in _prog_cache:
        _prog_cache[key] = _build_program(cf, cb, tpc)
    return _prog_cache[key]


# --------------------------------------------------------------------------
# entry point
# --------------------------------------------------------------------------

def kernel(x_real, x_imag, W_real, W_imag, b_real, b_imag, edge_index):
    import time
    t0 = time.time()
    x_real = np.asarray(x_real, dtype=np.float32)
    x_imag = np.asarray(x_imag, dtype=np.float32)
    W_real = np.asarray(W_real, dtype=np.float32)
    W_imag = np.asarray(W_imag, dtype=np.float32)
    b_real = np.asarray(b_real, dtype=np.float32)
    b_imag = np.asarray(b_imag, dtype=np.float32)
    edge_index = np.asarray(edge_index)

    prep = _host_prep(x_real, x_imag, W_real, W_imag, b_real, b_imag, edge_index)
    t1 = time.time()
    tpc = prep["tpc"]
    nc = _get_program(prep["cf"], prep["cb"], tpc)
    t2 = time.time()

    in_maps = []
    for corei in range(NCORES):
        in_maps.append({
            "xrT": np.ascontiguousarray(prep["xrT"][:, corei * NPC:(corei + 1) * NPC]),
            "xiT": np.ascontiguousarray(prep["xiT"][:, corei * NPC:(corei + 1) * NPC]),
            "wA": prep["wA"], "wB": prep["wB"], "wC": prep["wC"],
            "avec": prep["avec"][corei], "bvec": prep["bvec"][corei],
            "srcs": prep["src_all"][corei],
            "dstf": prep["dstf_all"][corei],
            "afac": prep["afac"][corei],
            "bfac": prep["bfac"][corei],
            "iota": prep["iota"],
        })

    if _SIM:
        from concourse import bass_interp
        outs = []
        for corei in range(NCORES):
            sim = bass_interp.CoreSim(nc)
            for k, v in in_maps[corei].items():
                sim.tensor(k)[:] = v
            sim.simulate()
            outs.append(sim.tensor("out").copy())
    else:
        t3 = time.time()
        res = bass_utils.run_bass_kernel_spmd(
            nc, in_maps, core_ids=list(range(NCORES)))
        _last_info["exec_wall_s"] = time.time() - t3
        _last_info["nc"] = nc
        _last_info["in_maps"] = in_maps
        outs = [r["out"] for r in res.results]
    _last_info["prep_s"] = t1 - t0
    _last_info["compile_s"] = t2 - t1

    full = np.concatenate(outs, axis=0).astype(np.float32)  # [NPAD, 256]
    out_nodes = full[prep["gslot"]]                         # [n, 256]
    total_real = out_nodes[:, :P] + prep["c1"][None, :]
    total_imag = out_nodes[:, P:] + prep["c2"][None, :]
    return total_real.astype(np.float32), total_imag.astype(np.float32)
